# revision 1
# baseline (speedup 1.0000x reference)
"""Trainium2 Bass kernel for fused MHA block (QKV -> masked softmax attention
-> out-proj -> residual -> LayerNorm), sharded over 8 NeuronCores.

Sharding: core c handles batch b=c//4 and query rows [512*r, 512*(r+1)) with
r=c%4. Each core computes QKV for its own 512 rows, AllGathers K^T and V
across the 4 cores of its batch, runs attention for its rows over all 16
heads (scores computed transposed [k, q] so no on-chip transposes are ever
needed), then out-projection + residual + LayerNorm natively.

Self-contained: hardcodes all shapes; only needs numpy/ml_dtypes/concourse.
"""

import numpy as np
import ml_dtypes

from concourse import bacc, bass_utils, mybir, tile
import concourse.bass as bass
from concourse.masks import make_identity

B, S, D = 2, 2048, 1024
H, DH = 16, 64
SL = 512  # per-core query-row shard
NCORES = 8
R = 4  # ranks per replica group (one batch)
GROUPS = [[0, 1, 2, 3], [4, 5, 6, 7]]

f32 = mybir.dt.float32
f32r = mybir.dt.float32r
bf16 = mybir.dt.bfloat16
AF = mybir.ActivationFunctionType
ALU = mybir.AluOpType

# matmul compute dtype for the fp32 data chain (float32r = full-rate PE)
fmm = f32r


def _build():
    nc = bacc.Bacc("TRN2", target_bir_lowering=False, debug=False,
                   num_devices=NCORES)

    xT = nc.dram_tensor("xT", [D, SL], fmm, kind="ExternalInput")
    wqkv = nc.dram_tensor("wqkv", [D, 3 * D], fmm, kind="ExternalInput")
    bq = nc.dram_tensor("bq", [128, 8], f32, kind="ExternalInput")
    bk = nc.dram_tensor("bk", [128, 8], f32, kind="ExternalInput")
    bv = nc.dram_tensor("bv", [1, D], f32, kind="ExternalInput")
    wout = nc.dram_tensor("wout", [D, D], fmm, kind="ExternalInput")
    maskT = nc.dram_tensor("maskT", [S, SL], bf16, kind="ExternalInput")
    xres = nc.dram_tensor("xres", [SL, D], f32, kind="ExternalInput")
    lng = nc.dram_tensor("lng", [1, D], f32, kind="ExternalInput")
    lnb = nc.dram_tensor("lnb", [1, D], f32, kind="ExternalInput")
    out = nc.dram_tensor("out", [SL, D], f32, kind="ExternalOutput")

    with tile.TileContext(nc) as tc:
        _body(tc, nc, xT, wqkv, bq, bk, bv, wout, maskT, xres, lng, lnb, out)
    nc.compile()
    return nc


def _body(tc, nc, xT, wqkv, bq, bk, bv, wout, maskT, xres, lng, lnb, out):
    with (
        tc.tile_pool(name="singles", bufs=1) as singles,
        tc.tile_pool(name="dpool", bufs=1, space="DRAM") as dpool,
    ):
        # ---- constants / long-lived tiles ----
        ident = singles.tile([128, 128], bf16)
        make_identity(nc, ident)
        bqs = singles.tile([128, 8], f32)
        nc.sync.dma_start(out=bqs, in_=bq.ap())
        bks = singles.tile([128, 8], f32)
        nc.sync.dma_start(out=bks, in_=bk.ap())
        bvb = singles.tile([128, D], f32)
        nc.sync.dma_start(out=bvb, in_=bv.ap().to_broadcast([128, D]))
        lngb = singles.tile([128, D], f32)
        nc.sync.dma_start(out=lngb, in_=lng.ap().to_broadcast([128, D]))
        lnbb = singles.tile([128, D], f32)
        nc.sync.dma_start(out=lnbb, in_=lnb.ap().to_broadcast([128, D]))
        epss = singles.tile([128, 1], f32)
        nc.vector.memset(epss, 1e-5)
        ones4 = singles.tile([128, 4], f32)
        nc.vector.memset(ones4, 1.0)
        maskT_sb = singles.tile([128, 16, SL], bf16)
        nc.sync.dma_start(
            out=maskT_sb, in_=maskT.ap().rearrange("(t p) q -> p t q", p=128)
        )
        xres_sb = singles.tile([128, 4, D], f32)
        nc.sync.dma_start(
            out=xres_sb, in_=xres.ap().rearrange("(t p) d -> p t d", p=128)
        )
        qT_sb = singles.tile([128, 8, SL], fmm)
        attnT_sb = singles.tile([128, 8, SL], fmm)
        y_sb = singles.tile([128, 4, D], f32)

        # collective bounce buffers: per k-chunk c (128 local k rows),
        # block 0 = kT cols chunk [1024, 128], block 1 = v rows chunk
        # [128, 1024], both flattened to 131072 elements.
        CH = D * 128
        kv_loc = [dpool.tile([2, CH], fmm, name=f"kvloc{c}") for c in range(4)]
        kv_ag = [dpool.tile([R, 2, CH], fmm, name=f"kvag{c}") for c in range(4)]
        dn_dram = dpool.tile([4, 4, SL], f32)

        # ---- phase 1: QKV projection for this core's 512 rows ----
        with (
            tc.tile_pool(name="qkv_w", bufs=3) as wpool,
            tc.tile_pool(name="qkv_sb", bufs=1) as qsb,
            tc.tile_pool(name="qkv_ps", bufs=8, space="PSUM") as qps,
        ):
            xT_sb = qsb.tile([128, 8, SL], fmm)
            nc.sync.dma_start(
                out=xT_sb, in_=xT.ap().rearrange("(t p) q -> p t q", p=128)
            )
            kT_sb = qsb.tile([128, 8, SL], fmm)
            v_sb = qsb.tile([128, 4, D], fmm)

            # --- K^T: out tiles pt=0..8 cover W cols [1024, 2048) ---
            ps_k = [qps.tile([128, SL], f32, name=f"psk{pt}", tag="qkvps")
                    for pt in range(8)]
            for kt in range(8):
                wt = wpool.tile([128, D], fmm, name=f"wtk{kt}", tag="w")
                nc.sync.dma_start(
                    out=wt, in_=wqkv.ap()[kt * 128:(kt + 1) * 128, D:2 * D]
                )
                for pt in range(8):
                    nc.tensor.matmul(
                        ps_k[pt], (wt[:, pt * 128:(pt + 1) * 128]),
                        (xT_sb[:, kt, :]), start=(kt == 0), stop=(kt == 7),
                    )
            for pt in range(8):
                nc.scalar.activation(
                    out=kT_sb[:, pt, :], in_=ps_k[pt], func=AF.Identity,
                    bias=bks[:, pt:pt + 1], scale=1.0,
                )
            for c in range(4):
                # kT cols chunk c: [1024, 128] row-major == (t p f) flat
                nc.sync.dma_start(
                    out=kv_loc[c][0, :].rearrange("(t p f) -> p t f",
                                                  p=128, f=128),
                    in_=kT_sb[:, :, c * 128:(c + 1) * 128],
                )

            # --- V: out tiles (st, nch) cover W cols [2048, 3072) ---
            ps_v = [qps.tile([128, SL], f32, name=f"psv{i}", tag="qkvps")
                    for i in range(8)]
            for kt in range(8):
                wt = wpool.tile([128, D], fmm, name=f"wtv{kt}", tag="w")
                nc.sync.dma_start(
                    out=wt, in_=wqkv.ap()[kt * 128:(kt + 1) * 128, 2 * D:3 * D]
                )
                for st in range(4):
                    for nch in range(2):
                        nc.tensor.matmul(
                            ps_v[st * 2 + nch],
                            (xT_sb[:, kt, st * 128:(st + 1) * 128]),
                            (wt[:, nch * 512:(nch + 1) * 512]),
                            start=(kt == 0), stop=(kt == 7),
                        )
            for st in range(4):
                for nch in range(2):
                    nc.vector.tensor_add(
                        out=v_sb[:, st, nch * 512:(nch + 1) * 512],
                        in0=ps_v[st * 2 + nch],
                        in1=bvb[:, nch * 512:(nch + 1) * 512],
                    )
                # v rows chunk st: [128, 1024] == (p f) flat
                nc.sync.dma_start(
                    out=kv_loc[st][1, :].rearrange("(p f) -> p f", p=128),
                    in_=v_sb[:, st, :],
                )

            # --- chunked AllGather of K^T+V across the batch's 4 cores ---
            for c in range(4):
                nc.gpsimd.collective_compute(
                    "AllGather", ALU.bypass, replica_groups=GROUPS,
                    ins=[kv_loc[c].opt()], outs=[kv_ag[c].opt()],
                )

            # --- Q^T: out tiles pt=0..8 cover W cols [0, 1024) ---
            ps_q = [qps.tile([128, SL], f32, name=f"psq{pt}", tag="qkvps")
                    for pt in range(8)]
            for kt in range(8):
                wt = wpool.tile([128, D], fmm, name=f"wtq{kt}", tag="w")
                nc.sync.dma_start(
                    out=wt, in_=wqkv.ap()[kt * 128:(kt + 1) * 128, 0:D]
                )
                for pt in range(8):
                    nc.tensor.matmul(
                        ps_q[pt], (wt[:, pt * 128:(pt + 1) * 128]),
                        (xT_sb[:, kt, :]), start=(kt == 0), stop=(kt == 7),
                    )
            for pt in range(8):
                nc.scalar.activation(
                    out=qT_sb[:, pt, :], in_=ps_q[pt], func=AF.Identity,
                    bias=bqs[:, pt:pt + 1], scale=1.0,
                )

        # ---- phase 2: attention (scores transposed [k, q]) ----
        with (
            tc.tile_pool(name="att_kv", bufs=3) as kvp,
            tc.tile_pool(name="att_pr", bufs=3) as prp,
            tc.tile_pool(name="att_sc", bufs=2) as scp,
            tc.tile_pool(name="att_ps", bufs=2, space="PSUM") as psp,
            tc.tile_pool(name="att_av", bufs=1, space="PSUM") as avp,
        ):
            for hg in range(4):
                avs = avp.tile([128, 4, 512], f32, name=f"avs{hg}", tag="av")
                it = 0
                for c in range(4):
                    for j in range(4):
                        kt = 4 * j + c  # global k-tile index (mask slab)
                        kts = kvp.tile([128, 2, 128], fmm,
                                       name=f"kts{hg}_{kt}", tag="kts")
                        nc.sync.dma_start(
                            out=kts,
                            in_=kv_ag[c][j, 0, :]
                            .rearrange("(t p f) -> p t f", p=128, f=128)
                            [:, 2 * hg:2 * hg + 2, :],
                        )
                        vts = kvp.tile([128, 4, 66], fmm,
                                       name=f"vts{hg}_{kt}", tag="vts")
                        nc.sync.dma_start(
                            out=vts[:, :, 1:65],
                            in_=kv_ag[c][j, 1, :]
                            .rearrange("(p o d) -> p o d", p=128, d=64)
                            [:, 4 * hg:4 * hg + 4, :],
                        )
                        nc.vector.tensor_copy(vts[:, :, 65:66], ones4)
                        for hp in range(2):
                            ps = psp.tile([128, 2, 512], f32,
                                          name=f"ps{hg}_{kt}_{hp}", tag="ps")
                            for i in range(2):
                                hh = 2 * hp + i
                                h = 4 * hg + hh
                                po = (hh % 2) * 64
                                nc.tensor.matmul(
                                    ps[:, i, :],
                                    (kts[po:po + 64, hh // 2, :]),
                                    (qT_sb[po:po + 64, h // 2, :]),
                                    start=True, stop=False,
                                )
                                nc.tensor.matmul(
                                    ps[:, i, :], ident, maskT_sb[:, kt, :],
                                    start=False, stop=True,
                                )
                            pr = prp.tile([128, 2, 512], fmm,
                                          name=f"pr{hg}_{kt}_{hp}", tag="pr")
                            nc.scalar.activation(out=pr, in_=ps, func=AF.Exp,
                                                 scale=0.125)
                            for i in range(2):
                                hh = 2 * hp + i
                                nc.tensor.matmul(
                                    avs[0:65, hh, :], (vts[:, hh, 1:66]),
                                    (pr[:, i, :]),
                                    start=(it == 0), stop=(it == 15),
                                )
                        it += 1
                # normalize: 1/denom broadcast via DRAM, scale, place
                rc = scp.tile([65, 4, 512], f32, name=f"rc{hg}", tag="rc")
                nc.vector.reciprocal(out=rc[64:65, :, :], in_=avs[64:65, :, :])
                nc.sync.dma_start(out=dn_dram[hg, :, :], in_=rc[64:65, :, :])
                rb = scp.tile([64, 4, 512], f32, name=f"rb{hg}", tag="rb")
                dsrc = dn_dram[hg, :, :]
                nc.sync.dma_start(
                    out=rb,
                    in_=bass.AP(tensor=dsrc.tensor, offset=dsrc.offset,
                                ap=[[0, 64]] + [list(p) for p in dsrc.ap]),
                )
                atn = scp.tile([64, 4, 512], fmm, name=f"atn{hg}", tag="atn")
                nc.vector.tensor_mul(out=atn, in0=avs[0:64, :, :], in1=rb)
                for hh in range(4):
                    h = 4 * hg + hh
                    po = (h % 2) * 64
                    nc.sync.dma_start(
                        out=attnT_sb[po:po + 64, h // 2, :],
                        in_=atn[:, hh, :],
                    )

        # ---- phase 3: out-projection + residual + LayerNorm ----
        with (
            tc.tile_pool(name="op_w", bufs=3) as wop,
            tc.tile_pool(name="op_ps", bufs=8, space="PSUM") as opps,
            tc.tile_pool(name="ln", bufs=4) as lnp,
        ):
            for nch in range(2):
                yps = [opps.tile([128, 512], f32, name=f"yps{nch}_{qt}",
                                 tag="yps") for qt in range(4)]
                for kt in range(8):
                    wot = wop.tile([128, 512], fmm, name=f"wot{nch}_{kt}",
                                   tag="wot")
                    nc.sync.dma_start(
                        out=wot,
                        in_=wout.ap()[kt * 128:(kt + 1) * 128,
                                      nch * 512:(nch + 1) * 512],
                    )
                    for qt in range(4):
                        nc.tensor.matmul(
                            yps[qt],
                            (attnT_sb[:, kt, qt * 128:(qt + 1) * 128]),
                            (wot), start=(kt == 0), stop=(kt == 7),
                        )
                for qt in range(4):
                    nc.vector.tensor_add(
                        out=y_sb[:, qt, nch * 512:(nch + 1) * 512],
                        in0=yps[qt],
                        in1=xres_sb[:, qt, nch * 512:(nch + 1) * 512],
                    )
            for qt in range(4):
                stats = lnp.tile([128, 2, 6], f32, name=f"st{qt}", tag="st")
                for i in range(2):
                    nc.vector.bn_stats(
                        out=stats[:, i, :], in_=y_sb[:, qt, i * 512:(i + 1) * 512]
                    )
                mv = lnp.tile([128, 2], f32, name=f"mv{qt}", tag="mv")
                nc.vector.bn_aggr(out=mv, in_=stats)
                nc.scalar.activation(
                    out=mv[:, 1:2], in_=mv[:, 1:2], func=AF.Sqrt,
                    bias=epss, scale=1.0,
                )
                nc.vector.reciprocal(out=mv[:, 1:2], in_=mv[:, 1:2])
                yt = lnp.tile([128, D], f32, name=f"yt{qt}", tag="yt")
                nc.vector.tensor_scalar(
                    out=yt, in0=y_sb[:, qt, :], scalar1=mv[:, 0:1],
                    scalar2=mv[:, 1:2], op0=ALU.subtract, op1=ALU.mult,
                )
                nc.vector.tensor_mul(out=yt, in0=yt, in1=lngb)
                nc.vector.tensor_add(out=yt, in0=yt, in1=lnbb)
                nc.sync.dma_start(
                    out=out.ap()[qt * 128:(qt + 1) * 128, :], in_=yt
                )


_NC_CACHE = None


def kernel(**inputs) -> np.ndarray:
    global _NC_CACHE
    x = np.ascontiguousarray(np.asarray(inputs["x"], dtype=np.float32))
    W_attn = np.ascontiguousarray(np.asarray(inputs["W_attn"], np.float32))
    b_attn = np.asarray(inputs["b_attn"], np.float32)
    W_out = np.ascontiguousarray(np.asarray(inputs["W_out"], np.float32))
    b_out = np.asarray(inputs["b_out"], np.float32)
    ln_g = np.asarray(inputs["ln_g"], np.float32)
    ln_b = np.asarray(inputs["ln_b"], np.float32)
    mask = np.asarray(inputs["mask"])

    if _NC_CACHE is None:
        _NC_CACHE = _build()
    nc = _NC_CACHE

    bqa = np.ascontiguousarray(b_attn[0:D].reshape(8, 128).T)
    bka = np.ascontiguousarray(b_attn[D:2 * D].reshape(8, 128).T)
    bva = np.ascontiguousarray(b_attn[2 * D:3 * D].reshape(1, D))
    in_maps = []
    for c in range(NCORES):
        b, r = divmod(c, R)
        rows = slice(SL * r, SL * (r + 1))
        xTl = np.ascontiguousarray(x[b, rows, :].T)
        mT = np.ascontiguousarray(mask[b, 0, rows, :].T.astype(np.float32))
        mT = (mT * np.float32(-1e9)).astype(ml_dtypes.bfloat16)
        xresl = np.ascontiguousarray(x[b, rows, :] + b_out[None, :])
        in_maps.append(dict(
            xT=xTl, wqkv=W_attn, bq=bqa, bk=bka, bv=bva, wout=W_out,
            maskT=mT, xres=xresl, lng=ln_g.reshape(1, D),
            lnb=ln_b.reshape(1, D),
        ))

    res = bass_utils.run_bass_kernel_spmd(nc, in_maps,
                                          core_ids=list(range(NCORES)))
    kernel.last_results = res

    full = np.empty((B, S, D), np.float32)
    for c in range(NCORES):
        b, r = divmod(c, R)
        full[b, SL * r:SL * (r + 1), :] = res.results[c]["out"]
    return full


if __name__ == "__main__":
    rng = np.random.default_rng(0)
    ins = dict(
        x=rng.standard_normal((B, S, D), dtype=np.float32),
        W_attn=rng.standard_normal((D, 3 * D), dtype=np.float32) / 32,
        b_attn=np.zeros(3 * D, np.float32),
        W_out=rng.standard_normal((D, D), dtype=np.float32) / 32,
        b_out=np.zeros(D, np.float32),
        ln_g=np.ones(D, np.float32),
        ln_b=np.zeros(D, np.float32),
        mask=rng.integers(0, 5, (B, 1, S, S)) == 0,
    )
    y = kernel(**ins)
    print("ok", y.shape, y.dtype)



# revision 4
# speedup vs baseline: 1.2959x; 1.2959x over previous
"""Trainium2 Bass kernel for fused MHA block (QKV -> masked softmax attention
-> out-proj -> residual -> LayerNorm), sharded over 8 NeuronCores.

Sharding: core c handles batch b=c//4 and query rows [512*r, 512*(r+1)) with
r=c%4. Each core computes QKV for its own 512 rows, AllGathers K^T and V
(bf16, chunked, overlapped with V/Q projection) across the 4 cores of its
batch, runs attention for its rows over all 16 heads (scores computed
transposed [k, q]; mask applied as a 0/1 multiply after exp), then
out-projection + residual + LayerNorm natively.

Self-contained: hardcodes all shapes; only needs numpy/ml_dtypes/concourse.
"""

import numpy as np
import ml_dtypes

from concourse import bacc, bass_utils, mybir, tile
import concourse.bass as bass

B, S, D = 2, 2048, 1024
H, DH = 16, 64
SL = 512  # per-core query-row shard
NCORES = 8
R = 4  # ranks per replica group (one batch)
GROUPS = [[0, 1, 2, 3], [4, 5, 6, 7]]

f32 = mybir.dt.float32
bf16 = mybir.dt.bfloat16
AF = mybir.ActivationFunctionType
ALU = mybir.AluOpType

CHK = 8 * 128 * 128   # K^T chunk [1024, 128] flat (t p f)
CHV = 128 * 1024      # V chunk [128, 1024] flat (p f)


def _build():
    nc = bacc.Bacc("TRN2", target_bir_lowering=False, debug=False,
                   num_devices=NCORES)

    xT = nc.dram_tensor("xT", [D, SL], bf16, kind="ExternalInput")
    wq = nc.dram_tensor("wq", [D, D], bf16, kind="ExternalInput")
    wk = nc.dram_tensor("wk", [D, D], bf16, kind="ExternalInput")
    wv = nc.dram_tensor("wv", [D, D], bf16, kind="ExternalInput")
    bq = nc.dram_tensor("bq", [128, 8], f32, kind="ExternalInput")
    bk = nc.dram_tensor("bk", [128, 8], f32, kind="ExternalInput")
    bv = nc.dram_tensor("bv", [1, D], f32, kind="ExternalInput")
    wout2 = nc.dram_tensor("wout2", [64, 16, D], bf16, kind="ExternalInput")
    maskT = nc.dram_tensor("maskT", [S, SL], bf16, kind="ExternalInput")
    xres = nc.dram_tensor("xres", [SL, D], f32, kind="ExternalInput")
    lng = nc.dram_tensor("lng", [1, D], f32, kind="ExternalInput")
    lnb = nc.dram_tensor("lnb", [1, D], f32, kind="ExternalInput")
    out = nc.dram_tensor("out", [SL, D], f32, kind="ExternalOutput")

    with nc.allow_low_precision("bf16 softmax denom reciprocal, tol 2e-2"):
        with tile.TileContext(nc) as tc:
            _body(tc, nc, xT, wq, wk, wv, bq, bk, bv, wout2, maskT, xres,
                  lng, lnb, out)
    nc.compile()
    return nc


def _body(tc, nc, xT, wq, wk, wv, bq, bk, bv, wout2, maskT, xres, lng, lnb,
          out):
    with (
        tc.tile_pool(name="singles", bufs=1) as singles,
        tc.tile_pool(name="dpool", bufs=1, space="DRAM") as dpool,
    ):
        # ---- long-lived tiles ----
        bqs = singles.tile([128, 8], f32)
        nc.sync.dma_start(out=bqs, in_=bq.ap())
        bks = singles.tile([128, 8], f32)
        nc.sync.dma_start(out=bks, in_=bk.ap())
        bvb = singles.tile([128, D], f32)
        nc.sync.dma_start(out=bvb, in_=bv.ap().to_broadcast([128, D]))
        lngb = singles.tile([128, D], f32)
        nc.sync.dma_start(out=lngb, in_=lng.ap().to_broadcast([128, D]))
        lnbb = singles.tile([128, D], f32)
        nc.sync.dma_start(out=lnbb, in_=lnb.ap().to_broadcast([128, D]))
        epss = singles.tile([128, 1], f32)
        nc.vector.memset(epss, 1e-5)
        xres_sb = singles.tile([128, 4, D], f32)
        nc.sync.dma_start(
            out=xres_sb, in_=xres.ap().rearrange("(t p) d -> p t d", p=128)
        )
        qT_sb = singles.tile([128, 8, SL], bf16)
        attnT2 = singles.tile([64, 16, SL], bf16)

        # collective bounce buffers (all bf16)
        kvk_loc = [dpool.tile([CHK], bf16, name=f"kvkl{c}") for c in range(4)]
        kvk_ag = [dpool.tile([R, CHK], bf16, name=f"kvka{c}") for c in range(4)]
        kvv_loc = [dpool.tile([CHV], bf16, name=f"kvvl{c}") for c in range(4)]
        kvv_ag = [dpool.tile([R, CHV], bf16, name=f"kvva{c}") for c in range(4)]
        dn_dram = dpool.tile([8, 2, SL], bf16)

        with tc.tile_pool(name="prep", bufs=1) as prep:
            # gathered K^T per chunk: [p(d), t(d-tile), j(rank), f(k)]
            kt_c = [prep.tile([128, 8, 4, 128], bf16, name=f"ktc{c}")
                    for c in range(4)]
            # gathered V per chunk: [p(k), j(rank), h, d+1] (col 64 = ones)
            v_c = [prep.tile([128, 4, 16, 65], bf16, name=f"vc{c}")
                   for c in range(4)]
            maskkeep = prep.tile([128, 16, SL], bf16)
            nc.sync.dma_start(
                out=maskkeep,
                in_=maskT.ap().rearrange("(t p) q -> p t q", p=128),
            )
            for c in range(4):
                nc.vector.memset(v_c[c][:, :, :, 64:65], 1.0)

            # ---- phase 1: QKV projection for this core's 512 rows ----
            with (
                tc.tile_pool(name="qkv_w", bufs=3) as wpool,
                tc.tile_pool(name="qkv_sb", bufs=1) as qsb,
                tc.tile_pool(name="qkv_ps", bufs=8, space="PSUM") as qps,
            ):
                xT_sb = qsb.tile([128, 8, SL], bf16)
                nc.sync.dma_start(
                    out=xT_sb, in_=xT.ap().rearrange("(t p) q -> p t q", p=128)
                )
                kT_sb = qsb.tile([128, 8, SL], bf16)
                v_sb = qsb.tile([128, 4, D], bf16)

                # --- K^T ---
                ps_k = [qps.tile([128, SL], f32, name=f"psk{pt}", tag="qkvps")
                        for pt in range(8)]
                for kt in range(8):
                    wt = wpool.tile([128, D], bf16, name=f"wtk{kt}", tag="w")
                    nc.sync.dma_start(
                        out=wt, in_=wk.ap()[kt * 128:(kt + 1) * 128, :]
                    )
                    for pt in range(8):
                        nc.tensor.matmul(
                            ps_k[pt], (wt[:, pt * 128:(pt + 1) * 128]),
                            (xT_sb[:, kt, :]), start=(kt == 0), stop=(kt == 7),
                        )
                for pt in range(8):
                    nc.scalar.activation(
                        out=kT_sb[:, pt, :], in_=ps_k[pt], func=AF.Identity,
                        bias=bks[:, pt:pt + 1], scale=1.0,
                    )
                for c in range(4):
                    nc.sync.dma_start(
                        out=kvk_loc[c][:].rearrange("(t p f) -> p t f",
                                                    p=128, f=128),
                        in_=kT_sb[:, :, c * 128:(c + 1) * 128],
                    )
                # AllGather K chunks (overlaps V/Q projection below)
                for c in range(4):
                    nc.gpsimd.collective_compute(
                        "AllGather", ALU.bypass, replica_groups=GROUPS,
                        ins=[kvk_loc[c].opt()], outs=[kvk_ag[c].opt()],
                    )

                # --- V ---
                ps_v = [qps.tile([128, SL], f32, name=f"psv{i}", tag="qkvps")
                        for i in range(8)]
                for kt in range(8):
                    wt = wpool.tile([128, D], bf16, name=f"wtv{kt}", tag="w")
                    nc.sync.dma_start(
                        out=wt, in_=wv.ap()[kt * 128:(kt + 1) * 128, :]
                    )
                    for st in range(4):
                        for nch in range(2):
                            nc.tensor.matmul(
                                ps_v[st * 2 + nch],
                                (xT_sb[:, kt, st * 128:(st + 1) * 128]),
                                (wt[:, nch * 512:(nch + 1) * 512]),
                                start=(kt == 0), stop=(kt == 7),
                            )
                for st in range(4):
                    for nch in range(2):
                        nc.vector.tensor_add(
                            out=v_sb[:, st, nch * 512:(nch + 1) * 512],
                            in0=ps_v[st * 2 + nch],
                            in1=bvb[:, nch * 512:(nch + 1) * 512],
                        )
                    nc.sync.dma_start(
                        out=kvv_loc[st][:].rearrange("(p f) -> p f", p=128),
                        in_=v_sb[:, st, :],
                    )
                for c in range(4):
                    nc.gpsimd.collective_compute(
                        "AllGather", ALU.bypass, replica_groups=GROUPS,
                        ins=[kvv_loc[c].opt()], outs=[kvv_ag[c].opt()],
                    )

                # --- Q^T (runs on PE while the gathers fly) ---
                ps_q = [qps.tile([128, SL], f32, name=f"psq{pt}", tag="qkvps")
                        for pt in range(8)]
                for kt in range(8):
                    wt = wpool.tile([128, D], bf16, name=f"wtq{kt}", tag="w")
                    nc.sync.dma_start(
                        out=wt, in_=wq.ap()[kt * 128:(kt + 1) * 128, :]
                    )
                    for pt in range(8):
                        nc.tensor.matmul(
                            ps_q[pt], (wt[:, pt * 128:(pt + 1) * 128]),
                            (xT_sb[:, kt, :]), start=(kt == 0), stop=(kt == 7),
                        )
                for pt in range(8):
                    nc.scalar.activation(
                        out=qT_sb[:, pt, :], in_=ps_q[pt], func=AF.Identity,
                        bias=bqs[:, pt:pt + 1], scale=1.0,
                    )

                # load gathered chunks into SBUF (issued on gpsimd DGE so the
                # Sync queue never head-blocks on collective completion)
                for c in range(4):
                    for j in range(4):
                        nc.gpsimd.dma_start(
                            out=kt_c[c][:, :, j, :],
                            in_=kvk_ag[c][j, :].rearrange(
                                "(t p f) -> p t f", p=128, f=128),
                        )
                for c in range(4):
                    for j in range(4):
                        nc.gpsimd.dma_start(
                            out=v_c[c][:, j, :, 0:64],
                            in_=kvv_ag[c][j, :].rearrange(
                                "(p h d) -> p h d", p=128, d=64),
                        )

            # ---- phase 2: attention, software-pipelined ----
            with (
                tc.tile_pool(name="att_et", bufs=3) as etp,
                tc.tile_pool(name="att_pr", bufs=4) as prp,
                tc.tile_pool(name="att_fl", bufs=2) as flp,
                tc.tile_pool(name="att_ps", bufs=3, space="PSUM") as psp,
                tc.tile_pool(name="att_av", bufs=1, space="PSUM") as avp,
            ):
                order = [(c, j) for c in range(4) for j in range(4)]
                pending = []   # (hg, idx, pr) awaiting AV issue
                norm_q = []    # (hg, raw, dnb) awaiting recip+normalize
                avs_cur = [None]

                def issue_front(hg, idx):
                    c, j = order[idx]
                    kt = 4 * j + c
                    ps = psp.tile([128, 2, SL], f32, name=f"ps{hg}_{idx}",
                                  tag="ps")
                    for i in range(2):
                        h = 2 * hg + i
                        po = (h % 2) * 64
                        nc.tensor.matmul(
                            ps[:, i, :],
                            (kt_c[c][po:po + 64, h // 2, j, :]),
                            (qT_sb[po:po + 64, h // 2, :]),
                            start=True, stop=True,
                        )
                    et = etp.tile([128, 2, SL], bf16, name=f"et{hg}_{idx}",
                                  tag="et")
                    nc.scalar.activation(out=et, in_=ps, func=AF.Exp,
                                         scale=0.125)
                    pr = prp.tile([128, 2, SL], bf16, name=f"pr{hg}_{idx}",
                                  tag="pr")
                    for i in range(2):
                        nc.vector.tensor_mul(
                            out=pr[:, i, :], in0=et[:, i, :],
                            in1=maskkeep[:, kt, :],
                        )
                    return pr

                def issue_av(hg, idx, pr):
                    c, j = order[idx]
                    if idx == 0:
                        avs_cur[0] = avp.tile([128, 2, SL], f32,
                                              name=f"avs{hg}", tag="avs")
                    avs = avs_cur[0]
                    for i in range(2):
                        h = 2 * hg + i
                        nc.tensor.matmul(
                            avs[0:65, i, :], (v_c[c][:, j, h, :]),
                            (pr[:, i, :]), start=(idx == 0), stop=(idx == 15),
                        )
                    if idx == 15:
                        raw = flp.tile([65, 2, SL], bf16, name=f"raw{hg}",
                                       tag="raw")
                        nc.scalar.activation(out=raw, in_=avs[0:65, :, :],
                                             func=AF.Identity)
                        nc.sync.dma_start(out=dn_dram[hg, :, :],
                                          in_=raw[64:65, :, :])
                        dnb = flp.tile([64, 2, SL], bf16, name=f"dnb{hg}",
                                       tag="dnb")
                        dsrc = dn_dram[hg, :, :]
                        nc.sync.dma_start(
                            out=dnb,
                            in_=bass.AP(tensor=dsrc.tensor, offset=dsrc.offset,
                                        ap=[[0, 64]] + [list(p)
                                                        for p in dsrc.ap]),
                        )
                        norm_q.append((hg, raw, dnb))

                def issue_norm():
                    hg, raw, dnb = norm_q.pop(0)
                    rec = flp.tile([64, 2, SL], bf16, name=f"rec{hg}",
                                   tag="rec")
                    nc.vector.reciprocal(out=rec, in_=dnb)
                    nc.vector.tensor_mul(
                        out=attnT2[:, 2 * hg:2 * hg + 2, :],
                        in0=raw[0:64, :, :], in1=rec,
                    )

                for hg in range(8):
                    for idx in range(16):
                        pr = issue_front(hg, idx)
                        pending.append((hg, idx, pr))
                        if len(pending) > 2:
                            issue_av(*pending.pop(0))
                        if idx == 8 and norm_q:
                            issue_norm()
                while pending:
                    issue_av(*pending.pop(0))
                while norm_q:
                    issue_norm()

        # ---- phase 3: out-projection + residual + LayerNorm ----
        with (
            tc.tile_pool(name="op_w", bufs=2) as wop,
            tc.tile_pool(name="op_sb", bufs=1) as osb,
            tc.tile_pool(name="op_ps", bufs=8, space="PSUM") as opps,
            tc.tile_pool(name="ln", bufs=4) as lnp,
        ):
            y_sb = osb.tile([128, 4, D], f32)
            for nch in range(2):
                wot = wop.tile([64, 16, 512], bf16, name=f"wot{nch}",
                               tag="wot")
                nc.sync.dma_start(
                    out=wot, in_=wout2.ap()[:, :, nch * 512:(nch + 1) * 512]
                )
                yps = [opps.tile([128, 512], f32, name=f"yps{nch}_{qt}",
                                 tag="yps") for qt in range(4)]
                for h in range(16):
                    for qt in range(4):
                        nc.tensor.matmul(
                            yps[qt],
                            (attnT2[:, h, qt * 128:(qt + 1) * 128]),
                            (wot[:, h, :]), start=(h == 0), stop=(h == 15),
                        )
                for qt in range(4):
                    nc.vector.tensor_add(
                        out=y_sb[:, qt, nch * 512:(nch + 1) * 512],
                        in0=yps[qt],
                        in1=xres_sb[:, qt, nch * 512:(nch + 1) * 512],
                    )
            for qt in range(4):
                stats = lnp.tile([128, 2, 6], f32, name=f"st{qt}", tag="st")
                for i in range(2):
                    nc.vector.bn_stats(
                        out=stats[:, i, :],
                        in_=y_sb[:, qt, i * 512:(i + 1) * 512],
                    )
                mv = lnp.tile([128, 2], f32, name=f"mv{qt}", tag="mv")
                nc.vector.bn_aggr(out=mv, in_=stats)
                nc.scalar.activation(
                    out=mv[:, 1:2], in_=mv[:, 1:2], func=AF.Sqrt,
                    bias=epss, scale=1.0,
                )
                nc.vector.reciprocal(out=mv[:, 1:2], in_=mv[:, 1:2])
                yt = lnp.tile([128, D], f32, name=f"yt{qt}", tag="yt")
                nc.vector.tensor_scalar(
                    out=yt, in0=y_sb[:, qt, :], scalar1=mv[:, 0:1],
                    scalar2=mv[:, 1:2], op0=ALU.subtract, op1=ALU.mult,
                )
                nc.vector.tensor_mul(out=yt, in0=yt, in1=lngb)
                nc.vector.tensor_add(out=yt, in0=yt, in1=lnbb)
                nc.sync.dma_start(
                    out=out.ap()[qt * 128:(qt + 1) * 128, :], in_=yt
                )


_NC_CACHE = None


def kernel(**inputs) -> np.ndarray:
    global _NC_CACHE
    x = np.ascontiguousarray(np.asarray(inputs["x"], dtype=np.float32))
    W_attn = np.asarray(inputs["W_attn"], np.float32)
    b_attn = np.asarray(inputs["b_attn"], np.float32)
    W_out = np.asarray(inputs["W_out"], np.float32)
    b_out = np.asarray(inputs["b_out"], np.float32)
    ln_g = np.asarray(inputs["ln_g"], np.float32)
    ln_b = np.asarray(inputs["ln_b"], np.float32)
    mask = np.asarray(inputs["mask"])

    if _NC_CACHE is None:
        _NC_CACHE = _build()
    nc = _NC_CACHE

    bqa = np.ascontiguousarray(b_attn[0:D].reshape(8, 128).T)
    bka = np.ascontiguousarray(b_attn[D:2 * D].reshape(8, 128).T)
    bva = np.ascontiguousarray(b_attn[2 * D:3 * D].reshape(1, D))
    wqa = np.ascontiguousarray(W_attn[:, 0:D]).astype(ml_dtypes.bfloat16)
    wka = np.ascontiguousarray(W_attn[:, D:2 * D]).astype(ml_dtypes.bfloat16)
    wva = np.ascontiguousarray(W_attn[:, 2 * D:3 * D]).astype(ml_dtypes.bfloat16)
    wo2 = np.ascontiguousarray(
        W_out.reshape(16, 64, D).transpose(1, 0, 2)
    ).astype(ml_dtypes.bfloat16)
    in_maps = []
    for c in range(NCORES):
        b, r = divmod(c, R)
        rows = slice(SL * r, SL * (r + 1))
        xTl = np.ascontiguousarray(x[b, rows, :].T).astype(ml_dtypes.bfloat16)
        mkeep = np.ascontiguousarray(
            (~mask[b, 0, rows, :]).T.astype(np.float32)
        ).astype(ml_dtypes.bfloat16)
        xresl = np.ascontiguousarray(x[b, rows, :] + b_out[None, :])
        in_maps.append(dict(
            xT=xTl, wq=wqa, wk=wka, wv=wva, bq=bqa, bk=bka, bv=bva,
            wout2=wo2, maskT=mkeep, xres=xresl, lng=ln_g.reshape(1, D),
            lnb=ln_b.reshape(1, D),
        ))

    res = bass_utils.run_bass_kernel_spmd(nc, in_maps,
                                          core_ids=list(range(NCORES)))
    kernel.last_results = res

    full = np.empty((B, S, D), np.float32)
    for c in range(NCORES):
        b, r = divmod(c, R)
        full[b, SL * r:SL * (r + 1), :] = res.results[c]["out"]
    return full


if __name__ == "__main__":
    rng = np.random.default_rng(0)
    ins = dict(
        x=rng.standard_normal((B, S, D), dtype=np.float32),
        W_attn=rng.standard_normal((D, 3 * D), dtype=np.float32) / 32,
        b_attn=np.zeros(3 * D, np.float32),
        W_out=rng.standard_normal((D, D), dtype=np.float32) / 32,
        b_out=np.zeros(D, np.float32),
        ln_g=np.ones(D, np.float32),
        ln_b=np.zeros(D, np.float32),
        mask=rng.integers(0, 5, (B, 1, S, S)) == 0,
    )
    y = kernel(**ins)
    print("ok", y.shape, y.dtype)


# revision 6
# speedup vs baseline: 1.4906x; 1.1503x over previous
"""Trainium2 Bass kernel for fused MHA block (QKV -> masked softmax attention
-> out-proj -> residual -> LayerNorm), sharded over 8 NeuronCores.

Sharding: core c handles batch b=c//4 and query rows [512*r, 512*(r+1)) with
r=c%4. Each core computes QKV for its own 512 rows, AllGathers K^T and V
(bf16, one collective each, overlapped with V/Q projection) across the 4
cores of its batch, runs attention for its rows over all 16 heads (scores
computed transposed [k, q]; mask applied as a 0/1 multiply after exp), then
out-projection + residual + LayerNorm natively.

Self-contained: hardcodes all shapes; only needs numpy/ml_dtypes/concourse.
"""

import numpy as np
import ml_dtypes

from concourse import bacc, bass_utils, mybir, tile
import concourse.bass as bass

B, S, D = 2, 2048, 1024
H, DH = 16, 64
SL = 512  # per-core query-row shard
NCORES = 8
R = 4  # ranks per replica group (one batch)
GROUPS = [[0, 1, 2, 3], [4, 5, 6, 7]]

f32 = mybir.dt.float32
bf16 = mybir.dt.bfloat16
AF = mybir.ActivationFunctionType
ALU = mybir.AluOpType

CHK = 8 * 128 * SL    # full local K^T [1024, 512] flat (t p f)
CHV = 128 * D         # V chunk [128, 1024] flat (p f)


def _build():
    nc = bacc.Bacc("TRN2", target_bir_lowering=False, debug=False,
                   num_devices=NCORES)

    xT = nc.dram_tensor("xT", [D, SL], bf16, kind="ExternalInput")
    wq = nc.dram_tensor("wq", [D, D], bf16, kind="ExternalInput")
    wk = nc.dram_tensor("wk", [D, D], bf16, kind="ExternalInput")
    wv = nc.dram_tensor("wv", [D, D], bf16, kind="ExternalInput")
    bq = nc.dram_tensor("bq", [128, 8], f32, kind="ExternalInput")
    bk = nc.dram_tensor("bk", [128, 8], f32, kind="ExternalInput")
    bv = nc.dram_tensor("bv", [1, D], f32, kind="ExternalInput")
    wout2 = nc.dram_tensor("wout2", [64, 16, D], bf16, kind="ExternalInput")
    maskT = nc.dram_tensor("maskT", [S, SL], bf16, kind="ExternalInput")
    xres = nc.dram_tensor("xres", [SL, D], f32, kind="ExternalInput")
    lng = nc.dram_tensor("lng", [1, D], f32, kind="ExternalInput")
    lnb = nc.dram_tensor("lnb", [1, D], f32, kind="ExternalInput")
    out = nc.dram_tensor("out", [SL, D], f32, kind="ExternalOutput")

    with nc.allow_low_precision("bf16 softmax denom reciprocal, tol 2e-2"):
        with tile.TileContext(nc) as tc:
            _body(tc, nc, xT, wq, wk, wv, bq, bk, bv, wout2, maskT,
                  xres, lng, lnb, out)
    nc.compile()
    return nc


def _body(tc, nc, xT, wq, wk, wv, bq, bk, bv, wout2, maskT, xres, lng,
          lnb, out):
    with (
        tc.tile_pool(name="singles", bufs=1) as singles,
        tc.tile_pool(name="dpool", bufs=1, space="DRAM") as dpool,
    ):
        # ---- long-lived tiles ----
        bqs = singles.tile([128, 8], f32)
        nc.sync.dma_start(out=bqs, in_=bq.ap())
        bks = singles.tile([128, 8], f32)
        nc.sync.dma_start(out=bks, in_=bk.ap())
        bvb = singles.tile([128, D], f32)
        nc.sync.dma_start(out=bvb, in_=bv.ap().to_broadcast([128, D]))
        lngb = singles.tile([128, D], f32)
        nc.sync.dma_start(out=lngb, in_=lng.ap().to_broadcast([128, D]))
        lnbb = singles.tile([128, D], f32)
        nc.sync.dma_start(out=lnbb, in_=lnb.ap().to_broadcast([128, D]))
        epss = singles.tile([128, 1], f32)
        nc.vector.memset(epss, 1e-5)
        xres_sb = singles.tile([128, 4, D], f32)
        nc.sync.dma_start(
            out=xres_sb, in_=xres.ap().rearrange("(t p) d -> p t d", p=128)
        )
        qT_sb = singles.tile([128, 8, SL], bf16)
        attnT2 = singles.tile([64, 16, SL], bf16)

        # collective bounce buffers (all bf16)
        kvk_loc = dpool.tile([CHK], bf16)
        kvk_ag = dpool.tile([R, CHK], bf16)
        kvv_loc = dpool.tile([R, CHV], bf16)
        kvv_ag = dpool.tile([R, R, CHV], bf16)
        dn_dram = dpool.tile([8, 2, SL], bf16)
        dnr_dram = dpool.tile([8, 2, SL], bf16)

        with tc.tile_pool(name="prep", bufs=1) as prep:
            # local K^T [p(d), t(d-tile), k-local]; also the CCk payload
            kT_sb = prep.tile([128, 8, SL], bf16)
            # gathered K^T per chunk c (j != rank): [p(d), t, j, f(k)]
            kt_c = [prep.tile([128, 8, 4, 128], bf16, name=f"ktc{c}")
                    for c in range(4)]
            # gathered V per chunk: [p(k), j(rank), h, d+1] (col 64 = ones)
            v_c = [prep.tile([128, 4, 16, 65], bf16, name=f"vc{c}")
                   for c in range(4)]
            maskkeep = prep.tile([128, 16, SL], bf16)
            nc.sync.dma_start(
                out=maskkeep,
                in_=maskT.ap().rearrange("(t p) q -> p t q", p=128),
            )
            for c in range(4):
                nc.vector.memset(v_c[c][:, :, :, 64:65], 1.0)

            # ---- phase 1: QKV projection for this core's 512 rows ----
            with (
                tc.tile_pool(name="qkv_w", bufs=3) as wpool,
                tc.tile_pool(name="qkv_sb", bufs=1) as qsb,
                tc.tile_pool(name="qkv_ps", bufs=8, space="PSUM") as qps,
            ):
                xT_sb = qsb.tile([128, 8, SL], bf16)
                nc.sync.dma_start(
                    out=xT_sb, in_=xT.ap().rearrange("(t p) q -> p t q", p=128)
                )
                v_sb = qsb.tile([128, 4, D], bf16)

                # --- K^T ---
                ps_k = [qps.tile([128, SL], f32, name=f"psk{pt}", tag="qkvps")
                        for pt in range(8)]
                for kt in range(8):
                    wt = wpool.tile([128, D], bf16, name=f"wtk{kt}", tag="w")
                    nc.sync.dma_start(
                        out=wt, in_=wk.ap()[kt * 128:(kt + 1) * 128, :]
                    )
                    for pt in range(8):
                        nc.tensor.matmul(
                            ps_k[pt], (wt[:, pt * 128:(pt + 1) * 128]),
                            (xT_sb[:, kt, :]), start=(kt == 0), stop=(kt == 7),
                        )
                for pt in range(8):
                    nc.scalar.activation(
                        out=kT_sb[:, pt, :], in_=ps_k[pt], func=AF.Identity,
                        bias=bks[:, pt:pt + 1], scale=1.0,
                    )
                nc.sync.dma_start(
                    out=kvk_loc[:].rearrange("(t p f) -> p t f", p=128, f=SL),
                    in_=kT_sb,
                )
                nc.gpsimd.collective_compute(
                    "AllGather", ALU.bypass, replica_groups=GROUPS,
                    ins=[kvk_loc.opt()], outs=[kvk_ag.opt()],
                )

                # --- V (flushed straight into the local rank's v_c slots) ---
                ps_v = [qps.tile([128, SL], f32, name=f"psv{i}", tag="qkvps")
                        for i in range(8)]
                for kt in range(8):
                    wt = wpool.tile([128, D], bf16, name=f"wtv{kt}", tag="w")
                    nc.sync.dma_start(
                        out=wt, in_=wv.ap()[kt * 128:(kt + 1) * 128, :]
                    )
                    for st in range(4):
                        for nch in range(2):
                            nc.tensor.matmul(
                                ps_v[st * 2 + nch],
                                (xT_sb[:, kt, st * 128:(st + 1) * 128]),
                                (wt[:, nch * 512:(nch + 1) * 512]),
                                start=(kt == 0), stop=(kt == 7),
                            )
                for st in range(4):
                    for nch in range(2):
                        nc.vector.tensor_add(
                            out=v_sb[:, st, nch * 512:(nch + 1) * 512],
                            in0=ps_v[st * 2 + nch],
                            in1=bvb[:, nch * 512:(nch + 1) * 512],
                        )
                    nc.sync.dma_start(
                        out=kvv_loc[st, :].rearrange("(p f) -> p f", p=128),
                        in_=v_sb[:, st, :],
                    )
                nc.gpsimd.collective_compute(
                    "AllGather", ALU.bypass, replica_groups=GROUPS,
                    ins=[kvv_loc.opt()], outs=[kvv_ag.opt()],
                )

                # --- Q^T (runs on PE while the gathers fly) ---
                ps_q = [qps.tile([128, SL], f32, name=f"psq{pt}", tag="qkvps")
                        for pt in range(8)]
                for kt in range(8):
                    wt = wpool.tile([128, D], bf16, name=f"wtq{kt}", tag="w")
                    nc.sync.dma_start(
                        out=wt, in_=wq.ap()[kt * 128:(kt + 1) * 128, :]
                    )
                    for pt in range(8):
                        nc.tensor.matmul(
                            ps_q[pt], (wt[:, pt * 128:(pt + 1) * 128]),
                            (xT_sb[:, kt, :]), start=(kt == 0), stop=(kt == 7),
                        )
                for pt in range(8):
                    nc.scalar.activation(
                        out=qT_sb[:, pt, :], in_=ps_q[pt], func=AF.Identity,
                        bias=bqs[:, pt:pt + 1], scale=1.0,
                    )

                # load gathered chunks into SBUF; K loads on the gpsimd DGE,
                # V loads on sync, so neither queue head-blocks the other.
                for c in range(4):
                    for j in range(4):
                        nc.gpsimd.dma_start(
                            out=kt_c[c][:, :, j, :],
                            in_=kvk_ag[j, :].rearrange(
                                "(t p f) -> p t f", p=128, f=SL
                            )[:, :, c * 128:(c + 1) * 128],
                        )
                for c in range(4):
                    for j in range(4):
                        nc.sync.dma_start(
                            out=v_c[c][:, j, :, 0:64],
                            in_=kvv_ag[j, c, :].rearrange(
                                "(p h d) -> p h d", p=128, d=64),
                        )

            # ---- phase 2: attention, software-pipelined ----
            with (
                tc.tile_pool(name="att_et", bufs=3) as etp,
                tc.tile_pool(name="att_pr", bufs=4) as prp,
                tc.tile_pool(name="att_fl", bufs=2) as flp,
                tc.tile_pool(name="att_ps", bufs=2, space="PSUM") as psp,
                tc.tile_pool(name="att_av", bufs=2, space="PSUM") as avp,
            ):
                order = [(c, j) for c in range(4) for j in range(4)]
                pending = []   # (hg, idx, pr) awaiting AV issue
                norm_q = []    # (hg, raw, dnb) awaiting normalize
                avs_cur = [None]

                def kslab(c, j, po, ht):
                    return kt_c[c][po:po + 64, ht, j, :]

                def issue_front(hg, idx):
                    c, j = order[idx]
                    kt = 4 * j + c
                    ps = psp.tile([128, 2, SL], f32, name=f"ps{hg}_{idx}",
                                  tag="ps")
                    for i in range(2):
                        h = 2 * hg + i
                        po = (h % 2) * 64
                        nc.tensor.matmul(
                            ps[:, i, :], (kslab(c, j, po, h // 2)),
                            (qT_sb[po:po + 64, h // 2, :]),
                            start=True, stop=True,
                        )
                    et = etp.tile([128, 2, SL], bf16, name=f"et{hg}_{idx}",
                                  tag="et")
                    nc.scalar.activation(out=et, in_=ps, func=AF.Exp,
                                         scale=0.125)
                    pr = prp.tile([128, 2, SL], bf16, name=f"pr{hg}_{idx}",
                                  tag="pr")
                    for i in range(2):
                        nc.vector.tensor_mul(
                            out=pr[:, i, :], in0=et[:, i, :],
                            in1=maskkeep[:, kt, :],
                        )
                    return pr

                def issue_av(hg, idx, pr):
                    c, j = order[idx]
                    if idx == 0:
                        avs_cur[0] = avp.tile([128, 2, SL], f32,
                                              name=f"avs{hg}", tag="avs")
                    avs = avs_cur[0]
                    for i in range(2):
                        h = 2 * hg + i
                        nc.tensor.matmul(
                            avs[0:65, i, :], (v_c[c][:, j, h, :]),
                            (pr[:, i, :]), start=(idx == 0), stop=(idx == 15),
                        )
                    if idx == 15:
                        raw = flp.tile([65, 2, SL], bf16, name=f"raw{hg}",
                                       tag="raw")
                        nc.vector.tensor_copy(raw, avs[0:65, :, :])
                        nc.sync.dma_start(out=dn_dram[hg, :, :],
                                          in_=raw[64:65, :, :])
                        # reshape the 1024 denominators across 128 partitions,
                        # reciprocal there (cheap), bounce back broadcast
                        dnw = flp.tile([128, 8], bf16, name=f"dnw{hg}",
                                       tag="dnw")
                        nc.sync.dma_start(
                            out=dnw,
                            in_=dn_dram[hg, :, :].rearrange(
                                "i (a f) -> (i a) f", f=8),
                        )
                        rw = flp.tile([128, 8], bf16, name=f"rw{hg}", tag="rw")
                        nc.vector.reciprocal(out=rw, in_=dnw)
                        nc.sync.dma_start(
                            out=dnr_dram[hg, :, :].rearrange(
                                "i (a f) -> (i a) f", f=8),
                            in_=rw,
                        )
                        dnb = flp.tile([64, 2, SL], bf16, name=f"dnb{hg}",
                                       tag="dnb")
                        dsrc = dnr_dram[hg, :, :]
                        nc.sync.dma_start(
                            out=dnb,
                            in_=bass.AP(tensor=dsrc.tensor, offset=dsrc.offset,
                                        ap=[[0, 64]] + [list(p)
                                                        for p in dsrc.ap]),
                        )
                        norm_q.append((hg, raw, dnb))

                def issue_norm():
                    hg, raw, dnb = norm_q.pop(0)
                    nc.vector.tensor_mul(
                        out=attnT2[:, 2 * hg:2 * hg + 2, :],
                        in0=raw[0:64, :, :], in1=dnb,
                    )

                for hg in range(8):
                    for idx in range(16):
                        pr = issue_front(hg, idx)
                        pending.append((hg, idx, pr))
                        if len(pending) > 2:
                            issue_av(*pending.pop(0))
                        if idx == 8 and norm_q:
                            issue_norm()
                while pending:
                    issue_av(*pending.pop(0))
                while norm_q:
                    issue_norm()

        # ---- phase 3: out-projection + residual + LayerNorm ----
        with (
            tc.tile_pool(name="op_w", bufs=2) as wop,
            tc.tile_pool(name="op_sb", bufs=1) as osb,
            tc.tile_pool(name="op_ps", bufs=8, space="PSUM") as opps,
            tc.tile_pool(name="ln", bufs=4) as lnp,
        ):
            y_sb = osb.tile([128, 4, D], f32)
            for nch in range(2):
                wot = wop.tile([64, 16, 512], bf16, name=f"wot{nch}",
                               tag="wot")
                nc.sync.dma_start(
                    out=wot, in_=wout2.ap()[:, :, nch * 512:(nch + 1) * 512]
                )
                yps = [opps.tile([128, 512], f32, name=f"yps{nch}_{qt}",
                                 tag="yps") for qt in range(4)]
                for h in range(16):
                    for qt in range(4):
                        nc.tensor.matmul(
                            yps[qt],
                            (attnT2[:, h, qt * 128:(qt + 1) * 128]),
                            (wot[:, h, :]), start=(h == 0), stop=(h == 15),
                        )
                for qt in range(4):
                    nc.vector.tensor_add(
                        out=y_sb[:, qt, nch * 512:(nch + 1) * 512],
                        in0=yps[qt],
                        in1=xres_sb[:, qt, nch * 512:(nch + 1) * 512],
                    )
            for qt in range(4):
                stats = lnp.tile([128, 2, 6], f32, name=f"st{qt}", tag="st")
                for i in range(2):
                    nc.vector.bn_stats(
                        out=stats[:, i, :],
                        in_=y_sb[:, qt, i * 512:(i + 1) * 512],
                    )
                mv = lnp.tile([128, 2], f32, name=f"mv{qt}", tag="mv")
                nc.vector.bn_aggr(out=mv, in_=stats)
                nc.scalar.activation(
                    out=mv[:, 1:2], in_=mv[:, 1:2], func=AF.Sqrt,
                    bias=epss, scale=1.0,
                )
                nc.vector.reciprocal(out=mv[:, 1:2], in_=mv[:, 1:2])
                yt = lnp.tile([128, D], f32, name=f"yt{qt}", tag="yt")
                nc.vector.tensor_scalar(
                    out=yt, in0=y_sb[:, qt, :], scalar1=mv[:, 0:1],
                    scalar2=mv[:, 1:2], op0=ALU.subtract, op1=ALU.mult,
                )
                nc.gpsimd.tensor_mul(out=yt, in0=yt, in1=lngb)
                nc.gpsimd.tensor_add(out=yt, in0=yt, in1=lnbb)
                nc.sync.dma_start(
                    out=out.ap()[qt * 128:(qt + 1) * 128, :], in_=yt
                )


_NC_CACHE = None


def kernel(**inputs) -> np.ndarray:
    global _NC_CACHE
    x = np.ascontiguousarray(np.asarray(inputs["x"], dtype=np.float32))
    W_attn = np.asarray(inputs["W_attn"], np.float32)
    b_attn = np.asarray(inputs["b_attn"], np.float32)
    W_out = np.asarray(inputs["W_out"], np.float32)
    b_out = np.asarray(inputs["b_out"], np.float32)
    ln_g = np.asarray(inputs["ln_g"], np.float32)
    ln_b = np.asarray(inputs["ln_b"], np.float32)
    mask = np.asarray(inputs["mask"])

    if _NC_CACHE is None:
        _NC_CACHE = _build()
    nc = _NC_CACHE

    bqa = np.ascontiguousarray(b_attn[0:D].reshape(8, 128).T)
    bka = np.ascontiguousarray(b_attn[D:2 * D].reshape(8, 128).T)
    bva = np.ascontiguousarray(b_attn[2 * D:3 * D].reshape(1, D))
    wqa = np.ascontiguousarray(W_attn[:, 0:D]).astype(ml_dtypes.bfloat16)
    wka = np.ascontiguousarray(W_attn[:, D:2 * D]).astype(ml_dtypes.bfloat16)
    wva = np.ascontiguousarray(W_attn[:, 2 * D:3 * D]).astype(ml_dtypes.bfloat16)
    wo2 = np.ascontiguousarray(
        W_out.reshape(16, 64, D).transpose(1, 0, 2)
    ).astype(ml_dtypes.bfloat16)
    in_maps = []
    for c in range(NCORES):
        b, r = divmod(c, R)
        rows = slice(SL * r, SL * (r + 1))
        xTl = np.ascontiguousarray(x[b, rows, :].T).astype(ml_dtypes.bfloat16)
        mkeep = np.ascontiguousarray(
            (~mask[b, 0, rows, :]).T.astype(np.float32)
        ).astype(ml_dtypes.bfloat16)
        xresl = np.ascontiguousarray(x[b, rows, :] + b_out[None, :])
        in_maps.append(dict(
            xT=xTl, wq=wqa, wk=wka, wv=wva, bq=bqa, bk=bka, bv=bva,
            wout2=wo2, maskT=mkeep, xres=xresl, lng=ln_g.reshape(1, D),
            lnb=ln_b.reshape(1, D),
        ))

    res = bass_utils.run_bass_kernel_spmd(nc, in_maps,
                                          core_ids=list(range(NCORES)))
    kernel.last_results = res

    full = np.empty((B, S, D), np.float32)
    for c in range(NCORES):
        b, r = divmod(c, R)
        full[b, SL * r:SL * (r + 1), :] = res.results[c]["out"]
    return full


if __name__ == "__main__":
    rng = np.random.default_rng(0)
    ins = dict(
        x=rng.standard_normal((B, S, D), dtype=np.float32),
        W_attn=rng.standard_normal((D, 3 * D), dtype=np.float32) / 32,
        b_attn=np.zeros(3 * D, np.float32),
        W_out=rng.standard_normal((D, D), dtype=np.float32) / 32,
        b_out=np.zeros(D, np.float32),
        ln_g=np.ones(D, np.float32),
        ln_b=np.zeros(D, np.float32),
        mask=rng.integers(0, 5, (B, 1, S, S)) == 0,
    )
    y = kernel(**ins)
    print("ok", y.shape, y.dtype)
